# revision 4
# baseline (speedup 1.0000x reference)
"""Trainium2 Bass kernel (v6) for nn_Attention_48610439856262.

Gated attention block:
    qkv = x @ W_qkv ; gate = x @ W_gate ; s = e @ W_s (added to k)
    attn = softmax(q @ (k+s).T * D**-0.5) ; out = (attn @ v) * gate
    y = out @ W_proj + b_proj

Sharding (8 cores, tensor-parallel over heads): core c owns heads
{2c, 2c+1}.  Each core computes its 128 feature columns for q/k+s/v/gate,
runs attention for 2 heads, and writes the partial projection
y_c = gated_c @ W_proj[128c:128c+128, :]; the host sums partials + b_proj.

HW-measured evolution (R16/R32 amplified slope, 8 axon trn2 cores):
  427us  v2 baseline (spill/merge attnv, gpsimd normalization)
  408us  + row-tiled scores (h0/h1 tile_position pairs), jj-sections with
           single-pass psv accumulation (no spill/merge)
  284us  + normalization muls moved gpsimd->DVE (gpsimd ops cost ~2.5us
           each on HW; only partition_broadcast remains there)
  257us  + v computed transposed on the PE (stationary = x token block,
           moving = W_v) killing 64 DMA-transposes/rep; proj pairs with
           merged y DMAs fused into phase-A unit stream
  251-257us + split proj drains (frees pwork half earlier); attn@v DR
           matmuls lag exps by 2 mb-pairs; k+s chain split into 2 units.

Key HW lessons (trn2, this toolchain):
  * gpsimd (Pool) instructions cost ~2.5us each -> keep it to the one
    partition_broadcast the normalization needs.
  * interleaved PSUM accumulation groups (alternating banks per matmul)
    are ruinous; bank-contiguous 8-matmul chains are fine.
  * 2x row tiling (contraction 64) works: paired h0/h1 score matmuls at
    base partitions 0/64 run concurrently (256 mms in 43us vs 68us).
  * exp on ACT is dtype-independent 1 elem/lane/cycle: 131us/rep floor.
  * DMA-transpose + small-copy chains for v were ~57us of wall; computing
    vT directly on the PE (small free dim, LDW-bound) is far cheaper.
"""

import numpy as np

B, N, C, H, D = 2, 2048, 1024, 16, 64
T = B * N              # 4096 tokens
NCORES = 8
F = 128                # feature columns per core (2 heads x 64)
KC = C // 128          # 8 contraction chunks
TB = T // 128          # 32 token blocks
NB = N // 128          # 16 key blocks per sequence
SCALE = D ** -0.5

_cache: dict = {}


def _build_program(reps=1, stage=4, norm="dve", drain_act=False,
                   sp_lite=False, split_drain=True):
    import concourse.bacc as bacc
    import concourse.tile as tile
    from concourse import mybir

    f32 = mybir.dt.float32
    bf16 = mybir.dt.bfloat16
    fp8 = mybir.dt.float8e4
    DR = mybir.MatmulPerfMode.DoubleRow
    Exp = mybir.ActivationFunctionType.Exp

    nc = bacc.Bacc("TRN2", target_bir_lowering=False, debug=False,
                   num_devices=NCORES)

    xT = nc.dram_tensor("xT", [C, T], bf16, kind="ExternalInput").ap()
    eT = nc.dram_tensor("eT", [C, T], bf16, kind="ExternalInput").ap()
    wq = nc.dram_tensor("wq", [C, F], bf16, kind="ExternalInput").ap()
    wk = nc.dram_tensor("wk", [C, F], bf16, kind="ExternalInput").ap()
    wv = nc.dram_tensor("wv", [C, F], bf16, kind="ExternalInput").ap()
    ws = nc.dram_tensor("ws", [C, F], bf16, kind="ExternalInput").ap()
    wg = nc.dram_tensor("wg", [C, F], bf16, kind="ExternalInput").ap()
    wp = nc.dram_tensor("wp", [F, C], bf16, kind="ExternalInput").ap()
    y = nc.dram_tensor("y", [T, C], bf16, kind="ExternalOutput").ap()

    with tile.TileContext(nc) as tc:
        with tc.tile_pool(name="persist", bufs=1) as persist, \
             tc.tile_pool(name="psum", bufs=1, space="PSUM") as psum, \
             tc.tile_pool(name="xa", bufs=4) as xa_pool, \
             tc.tile_pool(name="ea", bufs=4) as ea_pool, \
             tc.tile_pool(name="vt", bufs=3) as vt_pool, \
             tc.tile_pool(name="pt", bufs=3) as pt_pool, \
             tc.tile_pool(name="small", bufs=4) as small, \
             tc.tile_pool(name="yout", bufs=4) as y_pool:
            # Weights, contraction-chunked: [128 k-part, KC, 128 cols]
            w_sb = {}
            for name, src in (("wq", wq), ("wk", wk), ("wv", wv),
                              ("ws", ws), ("wg", wg)):
                t_ = persist.tile([128, KC, F], bf16, tag=name, name=f"w_{name}")
                nc.sync.dma_start(out=t_,
                                  in_=src.rearrange("(k p) f -> p k f", p=128))
                w_sb[name] = t_
            wp_sb = persist.tile([F, C], bf16, tag="wp")
            nc.sync.dma_start(out=wp_sb, in_=wp)

            qT_s = persist.tile([128, T], bf16, tag="qT")
            kpsT_s = persist.tile([128, T], bf16, tag="kpsT")
            gT_s = persist.tile([128, T], bf16, tag="gT")
            gatedT_s = persist.tile([128, T], bf16, tag="gatedT")
            # v_aug per head: [v(64) | ones | pad(15)] x 2 heads -> 160/blk.
            # attn@v out partitions 0-63 = head dims, partition 64 = softmax
            # denominator.  160B block stride satisfies DoubleRow step%16.
            v_s = persist.tile([128, TB, 160], fp8, tag="v")
            if sp_lite:
                nc.vector.memset(v_s, 0.02)
            nc.vector.memset(v_s[:, :, 64], 1.0)
            nc.vector.memset(v_s[:, :, 144], 1.0)

            # PSUM (8 banks): scores [128,2,512] x 2 bufs = 4 | psv0,psv1 = 2
            # | pwork [128,2,512] = 2 (phase-A acc + proj; one generation,
            # halves alternate via subtile dep tracking so proj pairs can
            # drain both banks in a single DVE copy)

            chunk_state = {}
            pwork = psum.tile([128, 2, 512], f32, tag="pework", name="pwork")
            pw_state = {"i": 0}

            def next_half():
                i = pw_state["i"]
                pw_state["i"] = i ^ 1
                return pwork[:, i, :]

            def phase_a_dma(t):
                """Prefetch chunk t's x/e slabs (issued ahead of the PE
                chains so the first matmul never waits on HBM)."""
                sl = slice(t * 512, (t + 1) * 512)
                xt = xa_pool.tile([128, KC, 512], bf16, tag="xt",
                                  name=f"xt{t}")
                nc.sync.dma_start(
                    out=xt,
                    in_=xT[:, sl].rearrange("(k p) t -> p k t", p=128))
                et = ea_pool.tile([128, KC, 512], bf16, tag="et",
                                  name=f"et{t}")
                nc.sync.dma_start(
                    out=et,
                    in_=eT[:, sl].rearrange("(k p) t -> p k t", p=128))
                chunk_state[t] = (xt, et)

            def phase_a_part(t, part):
                """One quarter of chunk t's projections.  Chains stay
                bank-contiguous (HW pays heavily for interleaved PSUM
                accumulation groups).  part 0: q | 1: k+s | 2: gate
                | 3: v + transposes."""
                sl = slice(t * 512, (t + 1) * 512)
                xt, et = chunk_state[t]
                if part == 3:
                    # v computed TRANSPOSED on the PE: stationary = x token
                    # block (lhsT), moving = W_v -> out [tokens, vcols] lands
                    # token-major, exactly v_s's layout.  Kills the 8 DMA
                    # transposes + 4 small copies per chunk.
                    acc4 = next_half()
                    for j in range(4):
                        jsl = slice(j * 128, (j + 1) * 128)
                        for k in range(KC):
                            nc.tensor.matmul(acc4[:, jsl],
                                             xt[:, k, jsl],
                                             w_sb["wv"][:, k, :],
                                             start=(k == 0),
                                             stop=(k == KC - 1))
                    nc.vector.tensor_copy(
                        v_s[:, t * 4:(t + 1) * 4, :].rearrange(
                            "p b (h c) -> p b h c", h=2)[:, :, :, 0:64],
                        acc4.rearrange("p (b h c) -> p b h c", b=4, h=2))
                    chunk_state.pop(t)
                    return
                if part == 10:      # k+s first half: x @ W_k (chain stays
                    acc = next_half()            # open in this pwork half)
                    for k in range(KC):
                        nc.tensor.matmul(acc, w_sb["wk"][:, k, :],
                                         xt[:, k, :],
                                         start=(k == 0), stop=False)
                    chunk_state[(t, "ks")] = acc
                    return
                if part == 11:      # k+s second half: += e @ W_s, drain
                    acc = chunk_state.pop((t, "ks"))
                    for k in range(KC):
                        nc.tensor.matmul(acc, w_sb["ws"][:, k, :],
                                         et[:, k, :],
                                         start=False, stop=(k == KC - 1))
                    if drain_act:
                        nc.scalar.copy(kpsT_s[:, sl], acc)
                    else:
                        nc.vector.tensor_copy(kpsT_s[:, sl], acc)
                    return
                wname, dst = {0: ("wq", qT_s), 2: ("wg", gT_s)}[part]
                acc = next_half()
                for k in range(KC):
                    nc.tensor.matmul(acc, w_sb[wname][:, k, :], xt[:, k, :],
                                     start=(k == 0), stop=(k == KC - 1))
                nc.vector.tensor_copy(dst[:, sl], acc)

            def proj_pair(tb2):
                """Projection of token blocks (2*tb2, 2*tb2+1); one merged
                y DMA per pair (halves the SP issue count)."""
                yt = y_pool.tile([128, 2, 1024], bf16, tag="ytb")
                for a in range(2):
                    tb = tb2 * 2 + a
                    pw_state["i"] = 0    # align so j maps to half j
                    pys = []
                    for j in range(2):
                        py = next_half()
                        nc.tensor.matmul(py,
                                         gatedT_s[:, tb * 128:(tb + 1) * 128],
                                         wp_sb[:, j * 512:(j + 1) * 512],
                                         start=True, stop=True)
                        pys.append(py)
                    if split_drain:
                        for j in range(2):
                            nc.vector.tensor_copy(
                                yt[:, a, j * 512:(j + 1) * 512], pys[j])
                    else:
                        # both halves drained in ONE DVE copy
                        nc.vector.tensor_copy(
                            yt[:, a, :].rearrange("p (j q) -> p j q", j=2),
                            pwork)
                nc.sync.dma_start(
                    out=y[tb2 * 256:(tb2 + 1) * 256, :].rearrange(
                        "(a p) c -> p a c", p=128),
                    in_=yt)

            def section(b, nh, jj, slot_work):
                """Attention for queries (b, nh*1024 + jj*512 .. +512), both
                heads, all 2048 keys.  Scores h0/h1 go out as tile_position
                (0,0)/(64,0) row-tile pairs (concurrent on the PE array);
                attn@v runs as two bank-contiguous 8-matmul DR chains after
                all 16 exps.  slot_work: callables interleaved into the
                section."""
                nsl = slice(b * N + nh * 1024 + jj * 512,
                            b * N + nh * 1024 + (jj + 1) * 512)
                psv = [psum.tile([65, 512], f32, tag=f"psv{h}",
                                 name=f"psv{h}") for h in range(2)] \
                    if stage >= 3 else None
                ptts = []
                # DR attn@v lags the exps by 2 mb-pairs so the PE never
                # stalls waiting for ACT to finish the current pair.
                LAG = 2
                for mbp in range(8 + LAG):
                    if mbp < 8:
                        for mi in range(2):
                            mb = mbp * 2 + mi
                            msl = slice(b * N + mb * 128,
                                        b * N + mb * 128 + 128)
                            ps = psum.tile([128, 2, 512], f32, tag="scores",
                                           bufs=2, name="scores")
                            nc.tensor.matmul(ps[:, 0, :], kpsT_s[0:64, msl],
                                             qT_s[0:64, nsl],
                                             start=True, stop=True)
                            nc.tensor.matmul(ps[:, 1, :],
                                             kpsT_s[64:128, msl],
                                             qT_s[64:128, nsl],
                                             start=True, stop=True)
                            if mi == 0:
                                ptts.append(
                                    pt_pool.tile([128, 2, 2, 512], fp8,
                                                 tag="pt", name="ptt",
                                                 bufs=3 + LAG))
                            nc.scalar.activation(ptts[mbp][:, mi, :, :], ps,
                                                 Exp, scale=SCALE)
                        if stage < 3:
                            sk = small.tile([1, 2, 2, 512], fp8, tag="sink")
                            nc.gpsimd.tensor_copy(sk, ptts[mbp][0:1])
                    if stage >= 3 and mbp >= LAG:
                        mbl = mbp - LAG
                        pr = b * NB + mbl * 2
                        for h in range(2):
                            nc.tensor.matmul(
                                psv[h],
                                v_s[:, pr:pr + 2, h * 80:h * 80 + 65],
                                ptts[mbl][:, :, h, :],
                                start=(mbl == 0), stop=(mbl == 7),
                                perf_mode=DR)
                    if mbp >= 1 and slot_work:
                        slot_work.pop(0)()
                if stage < 3:
                    while slot_work:
                        slot_work.pop(0)()
                    return
                # normalize + gate both heads
                for h in range(2):
                    hsl = slice(h * 64, h * 64 + 64)
                    sacc = small.tile([65, 512], f32, tag="sacc")
                    nc.vector.tensor_copy(sacc, psv[h])
                    if norm == "off":
                        # perf probe only: skip the division (wrong math)
                        if h == 0:
                            nc.vector.tensor_mul(gatedT_s[hsl, nsl],
                                                 sacc[0:64, :],
                                                 gT_s[hsl, nsl])
                        else:
                            tmpb = small.tile([128, 512], bf16, tag="tmp")
                            nc.vector.tensor_copy(tmpb[0:64, :],
                                                  sacc[0:64, :])
                            tmp2 = small.tile([128, 512], bf16, tag="tmp2")
                            nc.sync.dma_start(out=tmp2[64:128, :],
                                              in_=tmpb[0:64, :])
                            nc.vector.tensor_mul(gatedT_s[hsl, nsl],
                                                 tmp2[64:128, :],
                                                 gT_s[hsl, nsl])
                        continue
                    # partition_broadcast only works from partition 0 on HW,
                    # so DMA the denominator row down to partition 0 first.
                    d0 = small.tile([1, 512], f32, tag="d0")
                    nc.sync.dma_start(out=d0, in_=sacc[64:65, :])
                    rs = small.tile([1, 512], f32, tag="rs")
                    nc.vector.reciprocal(rs, d0)
                    rb = small.tile([64, 512], f32, tag="rb")
                    nc.gpsimd.partition_broadcast(rb, rs)
                    tmp = small.tile([128, 512], bf16, tag="tmp")
                    eng = nc.vector if norm == "dve" else nc.gpsimd
                    eng.tensor_mul(tmp[0:64, :], sacc[0:64, :], rb)
                    if h == 0:
                        eng.tensor_mul(gatedT_s[hsl, nsl], tmp[0:64, :],
                                       gT_s[hsl, nsl])
                    else:
                        tmp2 = small.tile([128, 512], bf16, tag="tmp2")
                        nc.sync.dma_start(out=tmp2[64:128, :],
                                          in_=tmp[0:64, :])
                        eng.tensor_mul(gatedT_s[hsl, nsl],
                                       tmp2[64:128, :], gT_s[hsl, nsl])
                if stage < 4:
                    sk = small.tile([1, 512], bf16, tag="sink2")
                    nc.vector.tensor_copy(sk, gatedT_s[0:1, nsl])
                while slot_work:
                    slot_work.pop(0)()

            # Steady-state schedule per rep (sections b0 S0-S3, b1 S4-S7):
            #   S0-S3 slack: phase A chunks 4-7 for THIS rep's b1
            #                + proj of PREV rep's b1 (tg 4-7)
            #   S4-S7 slack: phase A chunks 0-3 for NEXT rep's b0
            #                + proj of THIS rep's b0 (tg 0-3)
            def phase_a_units(chunks, proj_tb2s=()):
                """DMA prefetch runs two chunks ahead of the PE chains.
                Every other PE chain part is paired with one projection
                pair: the proj matmuls' PSUM drains hide under the
                adjacent chain's compute (and vice versa)."""
                units = [lambda t=chunks[0]: phase_a_dma(t),
                         lambda t=chunks[1]: phase_a_dma(t)]
                proj_tb2s = list(proj_tb2s)
                for i, t in enumerate(chunks):
                    if i + 2 < len(chunks):
                        units.append(
                            lambda t2=chunks[i + 2]: phase_a_dma(t2))
                    # ks is split (10/11) with its pwork half held open in
                    # between — proj pairs may only go after closed parts.
                    for p in (0, "proj", 10, 11, 2, "proj", 3):
                        if p == "proj":
                            if proj_tb2s:
                                units.append(lambda tb2=proj_tb2s.pop(0):
                                             proj_pair(tb2))
                        else:
                            units.append(
                                lambda t=t, p=p: phase_a_part(t, p))
                return units

            for _rep in range(reps):
                first = _rep == 0
                last = _rep == reps - 1
                if first:
                    for u in phase_a_units((0, 1, 2, 3)):
                        u()
                b0_work = phase_a_units(
                    (4, 5, 6, 7),
                    range(8, 16) if (not first and stage >= 4) else ())
                b1_work = []
                if not last:
                    b1_work += phase_a_units(
                        (0, 1, 2, 3),
                        range(0, 8) if stage >= 4 else ())
                elif stage >= 4:
                    b1_work += [lambda tb2=tb2: proj_pair(tb2)
                                for tb2 in range(0, 8)]
                secs = [(0, nh, jj) for nh in range(2) for jj in range(2)] + \
                       [(1, nh, jj) for nh in range(2) for jj in range(2)]
                for i, (b, nh, jj) in enumerate(secs):
                    work = b0_work if b == 0 else b1_work
                    k = i % 4
                    n_slots = -(-len(work) // (4 - k))
                    section(b, nh, jj, [work.pop(0)
                                        for _ in range(min(n_slots,
                                                           len(work)))])
            if stage >= 4:
                for tb2 in range(8, 16):
                    proj_pair(tb2)

    nc.compile()
    return nc


def _get_nc():
    if "nc" not in _cache:
        _cache["nc"] = _build_program()
    return _cache["nc"]


def _get_exec():
    """Compile once; cache a persistent sharded executable."""
    if "exec" in _cache:
        return _cache["exec"]
    import jax
    from jax.experimental.shard_map import shard_map
    from jax.sharding import Mesh, PartitionSpec
    from concourse import mybir
    from concourse.bass2jax import (_bass_exec_p, install_neuronx_cc_hook,
                                    partition_id_tensor)

    nc = _get_nc()
    install_neuronx_cc_hook()
    partition_name = (nc.partition_id_tensor.name
                      if nc.partition_id_tensor else None)
    in_names, out_names, out_avals = [], [], []
    for alloc in nc.m.functions[0].allocations:
        if not isinstance(alloc, mybir.MemoryLocationSet):
            continue
        name = alloc.memorylocations[0].name
        if alloc.kind == "ExternalInput":
            if name != partition_name:
                in_names.append(name)
        elif alloc.kind == "ExternalOutput":
            out_names.append(name)
            out_avals.append(jax.core.ShapedArray(
                tuple(alloc.tensor_shape), mybir.dt.np(alloc.dtype)))
    n_params, n_outs = len(in_names), len(out_names)
    bind_in_names = tuple(in_names + out_names +
                          ([partition_name] if partition_name else []))

    def _body(*args):
        operands = list(args)
        if partition_name is not None:
            operands.append(partition_id_tensor())
        outs = _bass_exec_p.bind(
            *operands,
            out_avals=tuple(out_avals),
            in_names=bind_in_names,
            out_names=tuple(out_names),
            lowering_input_output_aliases=(),
            sim_require_finite=True,
            sim_require_nnan=True,
            nc=nc,
        )
        return tuple(outs)

    devices = jax.devices()[:NCORES]
    mesh = Mesh(np.asarray(devices), ("core",))
    in_specs = (PartitionSpec("core"),) * (n_params + n_outs)
    out_specs = (PartitionSpec("core"),) * n_outs
    sharded = jax.jit(shard_map(_body, mesh=mesh, in_specs=in_specs,
                                out_specs=out_specs, check_rep=False),
                      keep_unused=True)
    zeros_dev = [
        jax.device_put(
            np.zeros((NCORES * a.shape[0], *a.shape[1:]), a.dtype),
            jax.sharding.NamedSharding(mesh, PartitionSpec("core")))
        for a in out_avals]
    y_shape = out_avals[out_names.index("y")].shape
    if y_shape[0] * NCORES == NCORES * C and y_shape == (C, T):
        reduce_fn = jax.jit(
            lambda a: a.reshape(NCORES, C, T).astype(jax.numpy.float32)
            .sum(axis=0).T)
    else:
        reduce_fn = jax.jit(
            lambda a: a.reshape(NCORES, T, C).astype(jax.numpy.float32)
            .sum(axis=0))
    ex = {"fn": sharded, "in_names": in_names, "out_names": out_names,
          "out_avals": out_avals, "mesh": mesh, "zeros_dev": zeros_dev,
          "spec": PartitionSpec("core"), "reduce": reduce_fn}
    _cache["exec"] = ex
    return ex


def _make_in_maps(x, e, W_qkv, W_s, W_gate, W_proj):
    import ml_dtypes
    bf = ml_dtypes.bfloat16
    xT = np.ascontiguousarray(
        np.asarray(x, np.float32).reshape(T, C).T).astype(bf)
    eT = np.ascontiguousarray(
        np.asarray(e, np.float32).reshape(T, C).T).astype(bf)
    in_maps = []
    for c in range(NCORES):
        fs = slice(F * c, F * (c + 1))
        in_maps.append({
            "xT": xT,
            "eT": eT,
            "wq": np.ascontiguousarray(W_qkv[:, fs]).astype(bf),
            "wk": np.ascontiguousarray(W_qkv[:, C:][:, fs]).astype(bf),
            "wv": np.ascontiguousarray(W_qkv[:, 2 * C:][:, fs]).astype(bf),
            "ws": np.ascontiguousarray(W_s[:, fs]).astype(bf),
            "wg": np.ascontiguousarray(W_gate[:, fs]).astype(bf),
            "wp": np.ascontiguousarray(W_proj[fs, :]).astype(bf),
        })
    return in_maps


def kernel(x, e, W_qkv, W_s, W_gate, W_proj, b_proj):
    ex = _get_exec()
    in_maps = _make_in_maps(np.asarray(x), np.asarray(e), np.asarray(W_qkv),
                            np.asarray(W_s), np.asarray(W_gate),
                            np.asarray(W_proj))
    concat_in = [
        np.concatenate([np.asarray(in_maps[c][name])
                        for c in range(NCORES)], axis=0)
        for name in ex["in_names"]]
    out = ex["fn"](*concat_in, *ex["zeros_dev"])
    iy = ex["out_names"].index("y")
    y_sum = np.asarray(ex["reduce"](out[iy]))   # cross-core partial sum, [T,C]
    y_sum = y_sum + np.asarray(b_proj, dtype=np.float32)
    return y_sum.reshape(B, N, C).astype(np.float32)
